# revision 1
# baseline (speedup 1.0000x reference)
"""AGNNet (2-layer AGNN conv + linear head) distributed over 8 trn2 NeuronCores.

Strategy (graph/data parallel, per sharding hint):
  - nodes sharded by dst range: core c owns nodes [c*6250, (c+1)*6250)
  - host groups edges by dst, degree-sorts each core's local nodes (undone on
    output), pads each 128-node tile's in-edge lists to a per-group slot count
  - device: L1 matmul from host-transposed x shard; build a bf16 row table
    [h(16), inv_norm, bias, pad] per node; AllGather the table; per node-tile
    indirect-DMA gather of neighbor rows + DVE/ACT softmax; PE accumulates the
    weighted sum over slots; second conv identical; head matmul + log_softmax.
  - a dedicated all-zero table row (bias column = -1e30) backs padding slots so
    they vanish in the softmax and contribute 0 to the weighted sum.
  - the kernel is split into sequential TileContexts: walrus tracks SWDGE
    (indirect DMA) ring occupancy in a cumulative 16-bit semaphore value, so
    each context must stay under ~60k gather descriptors; the context-exit
    drain+sem-clear resets the counter.
"""

import math
from contextlib import ExitStack
from dataclasses import dataclass

import numpy as np


@dataclass
class Cfg:
    n_cores: int = 8
    n_nodes: int = 50000
    f_in: int = 2000
    nh: int = 16
    nc_out: int = 20
    group: int = 4          # node tiles per conv gather group
    row: int = 20           # table row elems: 16 h, [16]=invn, [17]=bias, 18-19 pad
    P: int = 128
    max_ctx_descs: int = 55000   # SWDGE descriptor budget per TileContext

    @property
    def npc_raw(self) -> int:       # real nodes per core
        return self.n_nodes // self.n_cores

    @property
    def npc(self) -> int:           # padded nodes per core (mult of 128)
        return ((self.npc_raw + self.P - 1) // self.P) * self.P

    @property
    def ntiles(self) -> int:
        return self.npc // self.P

    @property
    def kchunks(self) -> int:       # 128-row chunks of the padded f_in
        return (self.f_in + self.P - 1) // self.P

    @property
    def f_pad(self) -> int:
        return self.kchunks * self.P

    @property
    def pad_gid(self) -> int:       # index of the all-zero table row
        return self.n_cores * self.npc

    def groups(self, kg: list[int]):
        """[(tile0, gsz, K)] for the conv gather groups."""
        out = []
        t = 0
        for K in kg:
            gsz = min(self.group, self.ntiles - t)
            out.append((t, gsz, K))
            t += gsz
        assert t == self.ntiles
        return out

    def chunks(self, kg: list[int]):
        """Split groups into runs whose gather descriptors fit one context."""
        runs, cur, cum = [], [], 0
        for item in self.groups(kg):
            _, gsz, K = item
            d = self.P * gsz * K
            assert d <= self.max_ctx_descs
            if cum + d > self.max_ctx_descs and cur:
                runs.append(cur)
                cur, cum = [], 0
            cur.append(item)
            cum += d
        if cur:
            runs.append(cur)
        return runs

    @property
    def n_groups(self) -> int:
        return (self.ntiles + self.group - 1) // self.group


FULL = Cfg()
NEG_BIG = -1.0e30


# --------------------------------------------------------------------------
# host-side preprocessing
# --------------------------------------------------------------------------

def preprocess(cfg: Cfg, x: np.ndarray, edge_index: np.ndarray):
    """Shard + layout transform. Returns (per_core list of dicts, kg, order_c)."""
    P, NPC, NPCR = cfg.P, cfg.npc, cfg.npc_raw
    n, C = cfg.n_nodes, cfg.n_cores

    src = edge_index[0].astype(np.int64)
    dst = edge_index[1].astype(np.int64)
    loop = np.arange(n, dtype=np.int64)
    src = np.concatenate([src, loop])
    dst = np.concatenate([dst, loop])

    core_of_dst = dst // NPCR

    # pass 1: per-core degree sort -> rank of each node within its core
    order_c, rank_c, deg_c = [], [], []
    for c in range(C):
        m = core_of_dst == c
        ld = dst[m] - c * NPCR
        deg = np.bincount(ld, minlength=NPCR)
        order = np.argsort(deg, kind="stable")      # ascending degree
        rank = np.empty(NPCR, np.int64)
        rank[order] = np.arange(NPCR)
        order_c.append(order)
        rank_c.append(rank)
        deg_c.append(deg)

    # new global id after per-core permutation + padding
    new_gid = np.empty(n, np.int64)
    for c in range(C):
        new_gid[c * NPCR:(c + 1) * NPCR] = c * NPC + rank_c[c]

    # per-group K (max in-degree in the group, shared across cores for SPMD)
    ngrp = cfg.n_groups
    kg = np.zeros(ngrp, np.int64)
    for c in range(C):
        degp = np.zeros(NPC, np.int64)
        degp[rank_c[c]] = deg_c[c]
        for g in range(ngrp):
            t0 = g * cfg.group
            gsz = min(cfg.group, cfg.ntiles - t0)
            kmax = degp[t0 * P:(t0 + gsz) * P].max()
            kg[g] = max(kg[g], kmax)
    kg = [int(max(k, 1)) for k in kg]
    kmax_all = max(kg)

    per_core = []
    for c in range(C):
        m = core_of_dst == c
        ld = dst[m] - c * NPCR
        gs = new_gid[src[m]]
        er = rank_c[c][ld]                       # dst rank of each edge
        eo = np.argsort(er, kind="stable")
        er_s = er[eo]
        gs_s = gs[eo]
        starts = np.zeros(NPC + 1, np.int64)
        np.cumsum(np.bincount(er_s, minlength=NPC), out=starts[1:])
        k_e = np.arange(er_s.size) - starts[er_s]
        M = np.full((NPC, kmax_all), cfg.pad_gid, np.int32)
        M[er_s, k_e] = gs_s.astype(np.int32)

        # idx layout: per group a [128, gsz*K] block, col = t_in_g*K + k
        blocks = []
        for (t0, gsz, K) in cfg.groups(kg):
            blk = M[t0 * P:(t0 + gsz) * P, :K]           # [gsz*128, K]
            blk = blk.reshape(gsz, P, K).transpose(1, 0, 2).reshape(P, gsz * K)
            blocks.append(blk)
        idx = np.ascontiguousarray(np.concatenate(blocks, axis=1))

        # x shard: permuted, padded, transposed, f-padded
        xs = x[c * NPCR:(c + 1) * NPCR][order_c[c]]      # [NPCR, f_in]
        xt = np.zeros((cfg.f_pad, NPC), np.float32)
        xt[:cfg.f_in, :NPCR] = xs.T
        per_core.append({"xt": np.ascontiguousarray(xt), "idx": idx})

    return per_core, kg, order_c


# --------------------------------------------------------------------------
# device kernel builder
# --------------------------------------------------------------------------

def build_kernel(cfg: Cfg, kg: list[int], phases: str = "ABCDE"):
    import concourse.bacc as bacc
    import concourse.tile as tile
    from concourse import bass, mybir
    from concourse.masks import make_identity

    P = cfg.P
    NH, NCO, ROW = cfg.nh, cfg.nc_out, cfg.row
    NPC, NT, KC = cfg.npc, cfg.ntiles, cfg.kchunks
    NFULL = cfg.n_cores * NPC
    f32 = mybir.dt.float32
    bf16 = mybir.dt.bfloat16
    i32 = mybir.dt.int32
    AX = mybir.AxisListType.X
    OP = mybir.AluOpType
    AF = mybir.ActivationFunctionType
    slot_cols = sum(gsz * K for (_, gsz, K) in cfg.groups(kg))

    nc = bacc.Bacc("TRN2", target_bir_lowering=False, debug=False,
                   num_devices=cfg.n_cores)

    xt_d = nc.dram_tensor("xt", [cfg.f_pad, NPC], f32, kind="ExternalInput")
    idx_d = nc.dram_tensor("idx", [P, slot_cols], i32, kind="ExternalInput")
    w1_d = nc.dram_tensor("w1p", [P, KC * NH], f32, kind="ExternalInput")
    b1_d = nc.dram_tensor("b1r", [P, NH], f32, kind="ExternalInput")
    w4_d = nc.dram_tensor("w4r", [NH, NCO], f32, kind="ExternalInput")
    b4_d = nc.dram_tensor("b4r", [P, NCO], f32, kind="ExternalInput")
    be_d = nc.dram_tensor("beta3r", [P, 1], f32, kind="ExternalInput")
    out_d = nc.dram_tensor("out", [NPC, NCO], f32, kind="ExternalOutput")

    tabA_l = nc.dram_tensor("tabA_l", [NPC, ROW], bf16)
    tabA_f = nc.dram_tensor("tabA_f", [NFULL + 2, ROW], bf16, addr_space="Shared")
    tabB_l = nc.dram_tensor("tabB_l", [NPC, ROW], bf16)
    tabB_f = nc.dram_tensor("tabB_f", [NFULL + 2, ROW], bf16, addr_space="Shared")

    # persistent SBUF (survives across TileContexts)
    def sb(name, shape, dtype):
        return nc.alloc_sbuf_tensor(name, list(shape), dtype)

    ident = sb("ident", [P, P], bf16)
    ident_f = sb("identf", [P, P], f32)
    zeros = sb("zeros", [P, P], f32)
    w1_sb = sb("w1sb", [P, KC * NH], f32)
    b1_sb = sb("b1sb", [P, NH], f32)
    w4_sb = sb("w4sb", [NH, NCO], f32)
    b4_sb = sb("b4sb", [P, NCO], f32)
    be_sb = sb("besb", [P, 1], f32)
    h_sb = sb("hsb", [P, NT * NH], f32)
    rows_sb = sb("rowssb", [P, NT * ROW], bf16)
    sq_sb = sb("sqsb", [P, NT * NH], f32)
    ss_sb = sb("sssb", [P, NT], f32)
    inv_sb = sb("invsb", [P, NT], f32)

    def epilogue_rows(tab_local):
        """h_sb -> inv norm -> rows_sb -> DMA to tab_local."""
        h3v = h_sb.ap().rearrange("p (t j) -> p t j", t=NT)
        nc.vector.tensor_mul(sq_sb.ap(), h_sb.ap(), h_sb.ap())
        nc.vector.reduce_sum(
            ss_sb.ap(), sq_sb.ap().rearrange("p (t j) -> p t j", t=NT), axis=AX)
        nc.scalar.sqrt(ss_sb.ap(), ss_sb.ap())
        nc.vector.tensor_scalar_add(ss_sb.ap(), ss_sb.ap(), 1.0e-12)
        nc.vector.reciprocal(inv_sb.ap(), ss_sb.ap())
        rv = rows_sb.ap().rearrange("p (t j) -> p t j", t=NT)
        nc.vector.tensor_copy(rv[:, :, 0:16], h3v)
        nc.vector.tensor_copy(rv[:, :, 16], inv_sb.ap())
        nc.vector.tensor_copy(rv[:, :, 17], zeros.ap()[:, 0:NT])
        nc.sync.dma_start(
            out=tab_local[:, :].rearrange("(t p) j -> p t j", p=P),
            in_=rv)

    def allgather(tab_local, tab_full):
        nc.gpsimd.collective_compute(
            "AllGather", OP.bypass,
            replica_groups=[list(range(cfg.n_cores))],
            ins=[tab_local.ap().opt()],
            outs=[tab_full.ap()[0:NFULL, :].opt()])

    # ---------------- phase A: consts, L1, table A, AG1 --------------------
    with tile.TileContext(nc) as tc:
        make_identity(nc, ident.ap())
        make_identity(nc, ident_f.ap())
        nc.gpsimd.memset(zeros.ap(), 0.0)
        nc.gpsimd.memset(rows_sb.ap(), 0.0)
        nc.sync.dma_start(out=w1_sb.ap(), in_=w1_d[:, :])
        nc.sync.dma_start(out=b1_sb.ap(), in_=b1_d[:, :])
        nc.sync.dma_start(out=w4_sb.ap(), in_=w4_d[:, :])
        nc.sync.dma_start(out=b4_sb.ap(), in_=b4_d[:, :])
        nc.sync.dma_start(out=be_sb.ap(), in_=be_d[:, :])
        with ExitStack() as ctx:
            const = ctx.enter_context(tc.tile_pool(name="pad", bufs=1))
            padrow = const.tile([1, ROW], bf16, tag="padrow")
            nc.gpsimd.memset(padrow[:], 0.0)
            nc.gpsimd.memset(padrow[:1, 17:18], NEG_BIG)
            nc.sync.dma_start(
                out=tabA_f[NFULL:NFULL + 2, :][None, :, :],
                in_=padrow[:1, None, :].to_broadcast([1, 2, ROW]))
            nc.sync.dma_start(
                out=tabB_f[NFULL:NFULL + 2, :][None, :, :],
                in_=padrow[:1, None, :].to_broadcast([1, 2, ROW]))

        with tc.tile_pool(name="l1x", bufs=3) as xp, \
             tc.tile_pool(name="l1p", bufs=4, space="PSUM") as pp:
            for t in range(NT):
                xw = xp.tile([P, KC * P], f32, tag="xw")
                src = xt_d[:, :].rearrange("(c p) m -> p c m", p=P)[:, :, t * P:(t + 1) * P]
                nc.sync.dma_start(
                    out=xw[:].rearrange("p (c j) -> p c j", c=KC), in_=src)
                ps = pp.tile([P, NH], f32, tag="l1ps")
                for c in range(KC):
                    nc.tensor.matmul(
                        out=ps[:], lhsT=xw[:, c * P:(c + 1) * P],
                        rhs=w1_sb.ap()[:, c * NH:(c + 1) * NH],
                        start=(c == 0), stop=(c == KC - 1))
                hsl = h_sb.ap()[:, t * NH:(t + 1) * NH]
                nc.vector.tensor_add(hsl, ps[:], b1_sb.ap())
                nc.vector.tensor_scalar_max(hsl, hsl, 0.0)
        epilogue_rows(tabA_l)
        allgather(tabA_l, tabA_f)

    # ---------------- conv layer (one TileContext per chunk) ---------------
    def conv(tab_local, tab_full, beta_ap_fn):
        off = 0
        t_seen = 0
        for run in cfg.chunks(kg):
            with tile.TileContext(nc) as tc:
                with tc.tile_pool(name="cv", bufs=2) as cv, \
                     tc.tile_pool(name="cvp", bufs=2, space="PSUM") as cvp:
                    for (t0, gsz, K) in run:
                        gk = gsz * K
                        idx_sb = cv.tile([P, gk], i32, tag="idx")
                        nc.sync.dma_start(
                            out=idx_sb[:], in_=idx_d[:, off:off + gk])
                        hs = cv.tile([P, gk * ROW], bf16, tag="hs")
                        # HW indirect DMA = ONE index per partition reading
                        # contiguous elems; one gather per slot column. The
                        # completion sem fires at descriptor-generation, so a
                        # trailing plain SWDGE DMA on the same ring provides a
                        # data-landed fence for the whole group.
                        for j in range(gk):
                            nc.gpsimd.indirect_dma_start(
                                out=hs[:, j * ROW:(j + 1) * ROW],
                                out_offset=None,
                                in_=tab_full.ap(),
                                in_offset=bass.IndirectOffsetOnAxis(
                                    ap=idx_sb[:, j:j + 1], axis=0),
                            )
                        guard = cv.tile([P, 4], i32, tag="guard")
                        flush = nc.gpsimd.dma_start(
                            out=guard[:], in_=idx_d[:, 0:4])
                        hd = cv.tile([P, gsz * ROW], bf16, tag="hd")
                        nc.sync.dma_start(
                            out=hd[:].rearrange("p (g j) -> p g j", g=gsz),
                            in_=tab_local[t0 * P:(t0 + gsz) * P, :].rearrange(
                                "(g p) j -> p g j", p=P))

                        hs4 = hs[:].rearrange("p (g k j) -> p g k j", g=gsz, k=K)
                        hd3 = hd[:].rearrange("p (g j) -> p g j", g=gsz)
                        tmp = cv.tile([P, gk * NH], bf16, tag="tmp")
                        tm4 = tmp[:].rearrange("p (g k j) -> p g k j", g=gsz, k=K)
                        mul1 = nc.vector.tensor_mul(
                            tm4, hs4[:, :, :, 0:16],
                            hd3[:, :, None, 0:16].to_broadcast([P, gsz, K, 16]))
                        bass._add_dep_helper(
                            mul1.ins, flush.ins, sync=True,
                            reason="hs consumer waits for gather ring drain")
                        alpha = cv.tile([P, gk], f32, tag="alpha")
                        al3 = alpha[:].rearrange("p (g k) -> p g k", g=gsz)
                        nc.vector.reduce_sum(
                            alpha[:],
                            tmp[:].rearrange("p (gk j) -> p gk j", j=NH), axis=AX)
                        nc.vector.tensor_mul(al3, al3, hs4[:, :, :, 16])
                        invd = cv.tile([P, gsz], f32, tag="invd")
                        beta_ap = beta_ap_fn()
                        if beta_ap is None:
                            nc.vector.tensor_copy(invd[:], hd3[:, :, 16])
                        else:
                            nc.vector.tensor_scalar_mul(
                                invd[:], hd3[:, :, 16], beta_ap)
                        nc.vector.tensor_mul(
                            al3, al3,
                            invd[:][:, :, None].to_broadcast([P, gsz, K]))
                        nc.vector.tensor_add(al3, al3, hs4[:, :, :, 17])
                        nm = cv.tile([P, gsz], f32, tag="nm")
                        nc.vector.reduce_max(nm[:], al3, axis=AX, negate=True)
                        nc.vector.tensor_add(
                            al3, al3,
                            nm[:][:, :, None].to_broadcast([P, gsz, K]))
                        e_bf = cv.tile([P, gk], bf16, tag="e")
                        nc.scalar.activation(e_bf[:], alpha[:], AF.Exp)
                        s = cv.tile([P, gsz], f32, tag="s")
                        nc.vector.reduce_sum(
                            s[:], e_bf[:].rearrange("p (g k) -> p g k", g=gsz),
                            axis=AX)
                        nc.vector.tensor_scalar_add(s[:], s[:], 1.0e-16)
                        r = cv.tile([P, gsz], f32, tag="r")
                        nc.vector.reciprocal(r[:], s[:])
                        coef = cv.tile([P, gk], bf16, tag="coef")
                        nc.vector.tensor_mul(
                            coef[:].rearrange("p (g k) -> p g k", g=gsz),
                            e_bf[:].rearrange("p (g k) -> p g k", g=gsz),
                            r[:][:, :, None].to_broadcast([P, gsz, K]))
                        tmp2 = cv.tile([P, gk * NH], bf16, tag="tmp2")
                        t24 = tmp2[:].rearrange("p (g k j) -> p g k j", g=gsz, k=K)
                        nc.vector.tensor_mul(
                            t24, hs4[:, :, :, 0:16],
                            coef[:].rearrange("p (g k) -> p g k", g=gsz)
                            [:, :, :, None].to_broadcast([P, gsz, K, 16]))
                        h2v = h_sb.ap()[:, t0 * NH:(t0 + gsz) * NH]
                        nc.vector.reduce_sum(
                            h2v,
                            tmp2[:].rearrange(
                                "p (g k j) -> p g j k", g=gsz, k=K),
                            axis=AX)
                        off += gk
                        t_seen += gsz
        assert t_seen == NT

    if "B" in phases:
        conv(tabA_l, tabA_f, lambda: None)

    # ---------------- phase C: table B + AG2 -------------------------------
    if "C" in phases:
        with tile.TileContext(nc) as tc:
            epilogue_rows(tabB_l)
            allgather(tabB_l, tabB_f)

    if "D" in phases:
        conv(tabB_l, tabB_f, lambda: be_sb.ap()[:, 0:1])

    # ---------------- head + log_softmax -----------------------------------
    if "G" in phases:
        # debug: gather group 0 from tabA_f and dump raw rows (as f32)
        (t0g, gszg, Kg) = cfg.groups(kg)[0]
        gkg = gszg * Kg
        with tile.TileContext(nc) as tc:
            with tc.tile_pool(name="dbg", bufs=1) as dbg:
                idx_sb = dbg.tile([P, gkg], i32, tag="idx")
                nc.sync.dma_start(out=idx_sb[:], in_=idx_d[:, 0:gkg])
                hs = dbg.tile([P, gkg * ROW], bf16, tag="hs")
                for j in range(gkg):
                    nc.gpsimd.indirect_dma_start(
                        out=hs[:, j * ROW:(j + 1) * ROW], out_offset=None,
                        in_=tabA_f.ap(),
                        in_offset=bass.IndirectOffsetOnAxis(
                            ap=idx_sb[:, j:j + 1], axis=0))
                guard = dbg.tile([P, 4], i32, tag="guard")
                flush = nc.gpsimd.dma_start(out=guard[:], in_=idx_d[:, 0:4])
                ncols = min(gkg * ROW, (NPC // P) * NCO * (NPC // P and 1) * 980)
                ncols = min(gkg * ROW, 980)
                hf = dbg.tile([P, ncols], f32, tag="hf")
                cp = nc.vector.tensor_copy(hf[:], hs[:, 0:ncols])
                bass._add_dep_helper(
                    cp.ins, flush.ins, sync=True, reason="debug drain")
                ov = out_d.ap().rearrange("(p q) j -> p (q j)", p=P)
                nc.sync.dma_start(out=ov[:, 0:ncols], in_=hf[:])
        nc.compile()
        return nc

    if "E" not in phases:
        # debug: dump h_sb (and inv_sb) into out
        with tile.TileContext(nc) as tc:
            ov = out_d.ap().rearrange("(p q) j -> p (q j)", p=P)
            nc.sync.dma_start(out=ov[:, 0:NT * NH], in_=h_sb.ap())
            nc.sync.dma_start(out=ov[:, NT * NH:NT * NH + NT], in_=inv_sb.ap())
        nc.compile()
        return nc

    with tile.TileContext(nc) as tc:
        with tc.tile_pool(name="hd", bufs=1) as hp, \
             tc.tile_pool(name="hdp", bufs=4, space="PSUM") as hpp:
            h3t = hp.tile([NH, NT * P], f32, tag="h3t")
            for t in range(NT):
                pst = hpp.tile([NH, P], f32, tag="pst")
                nc.tensor.transpose(
                    out=pst[:], in_=h_sb.ap()[:, t * NH:(t + 1) * NH],
                    identity=ident_f.ap())
                nc.vector.tensor_copy(h3t[:, t * P:(t + 1) * P], pst[:])
            lg = hp.tile([P, NT * NCO], f32, tag="lg")
            for t in range(NT):
                psl = hpp.tile([P, NCO], f32, tag="psl")
                nc.tensor.matmul(
                    out=psl[:], lhsT=h3t[:, t * P:(t + 1) * P], rhs=w4_sb.ap(),
                    start=True, stop=True)
                nc.vector.tensor_add(
                    lg[:, t * NCO:(t + 1) * NCO], psl[:], b4_sb.ap())
            lg3 = lg[:].rearrange("p (t j) -> p t j", t=NT)
            nm = hp.tile([P, NT], f32, tag="hnm")
            nc.vector.reduce_max(nm[:], lg3, axis=AX, negate=True)
            nc.vector.tensor_add(
                lg3, lg3, nm[:][:, :, None].to_broadcast([P, NT, NCO]))
            ex = hp.tile([P, NT * NCO], f32, tag="ex")
            nc.scalar.activation(ex[:], lg[:], AF.Exp)
            s = hp.tile([P, NT], f32, tag="hs_sum")
            nc.vector.reduce_sum(
                s[:], ex[:].rearrange("p (t j) -> p t j", t=NT), axis=AX)
            ls = hp.tile([P, NT], f32, tag="ls")
            nc.scalar.activation(ls[:], s[:], AF.Ln)
            nc.vector.tensor_sub(
                lg3, lg3, ls[:][:, :, None].to_broadcast([P, NT, NCO]))
            nc.sync.dma_start(
                out=out_d[:, :].rearrange("(t p) j -> p t j", p=P),
                in_=lg3)

    nc.compile()
    return nc


# --------------------------------------------------------------------------
# entry point
# --------------------------------------------------------------------------

def run(cfg: Cfg, inputs: dict, trace: bool = False):
    from concourse import bass_utils

    x = np.asarray(inputs["x"], np.float32)
    edge_index = np.asarray(inputs["edge_index"])
    W1 = np.asarray(inputs["W1"], np.float32)
    b1 = np.asarray(inputs["b1"], np.float32)
    W4 = np.asarray(inputs["W4"], np.float32)
    b4 = np.asarray(inputs["b4"], np.float32)
    beta3 = np.asarray(inputs["beta3"], np.float32)

    per_core, kg, order_c = preprocess(cfg, x, edge_index)
    nc = build_kernel(cfg, kg)

    P, KC, NH = cfg.P, cfg.kchunks, cfg.nh
    w1p = np.zeros((cfg.f_pad, NH), np.float32)
    w1p[:cfg.f_in] = W1
    w1p = np.ascontiguousarray(
        w1p.reshape(KC, P, NH).transpose(1, 0, 2).reshape(P, KC * NH))
    b1r = np.ascontiguousarray(np.broadcast_to(b1[None, :], (P, NH)))
    b4r = np.ascontiguousarray(np.broadcast_to(b4[None, :], (P, cfg.nc_out)))
    ber = np.ascontiguousarray(np.broadcast_to(beta3[None, :], (P, 1)))

    in_maps = []
    for c in range(cfg.n_cores):
        in_maps.append({
            "xt": per_core[c]["xt"],
            "idx": per_core[c]["idx"],
            "w1p": w1p, "b1r": b1r, "w4r": np.ascontiguousarray(W4),
            "b4r": b4r, "beta3r": ber,
        })

    res = bass_utils.run_bass_kernel_spmd(
        nc, in_maps, core_ids=list(range(cfg.n_cores)), trace=trace)

    out = np.empty((cfg.n_nodes, cfg.nc_out), np.float32)
    for c in range(cfg.n_cores):
        oc = np.asarray(res.results[c]["out"])[:cfg.npc_raw]
        out[c * cfg.npc_raw + order_c[c]] = oc
    return out, res


def kernel(**inputs) -> np.ndarray:
    out, _ = run(FULL, inputs, trace=False)
    return out



# revision 4
# speedup vs baseline: 1.0030x; 1.0030x over previous
"""AGNNet (2-layer AGNN conv + linear head) distributed over 8 trn2 NeuronCores.

Strategy (graph/data parallel, per sharding hint):
  - nodes sharded by dst range: core c owns nodes [c*6250, (c+1)*6250)
  - host groups edges by dst, degree-sorts each core's local nodes (undone on
    output), pads each 128-node tile's in-edge lists to a per-group slot count
  - device: L1 matmul from host-transposed x shard; build a bf16 row table
    [h(16), inv_norm, bias, pad] per node; AllGather the table; per node-tile
    indirect-DMA gather of neighbor rows + DVE/ACT softmax; PE accumulates the
    weighted sum over slots; second conv identical; head matmul + log_softmax.
  - a dedicated all-zero table row (bias column = -1e30) backs padding slots so
    they vanish in the softmax and contribute 0 to the weighted sum.
  - the kernel is split into sequential TileContexts: walrus tracks SWDGE
    (indirect DMA) ring occupancy in a cumulative 16-bit semaphore value, so
    each context must stay under ~60k gather descriptors; the context-exit
    drain+sem-clear resets the counter.
"""

import math
from contextlib import ExitStack
from dataclasses import dataclass

import numpy as np


@dataclass
class Cfg:
    n_cores: int = 8
    n_nodes: int = 50000
    f_in: int = 2000
    nh: int = 16
    nc_out: int = 20
    group: int = 4          # node tiles per conv gather group
    row: int = 20           # table row elems: 16 h, [16]=invn, [17]=bias, 18-19 pad
    P: int = 128
    max_ctx_descs: int = 55000   # SWDGE descriptor budget per TileContext

    @property
    def npc_raw(self) -> int:       # real nodes per core
        return self.n_nodes // self.n_cores

    @property
    def npc(self) -> int:           # padded nodes per core (mult of 128)
        return ((self.npc_raw + self.P - 1) // self.P) * self.P

    @property
    def ntiles(self) -> int:
        return self.npc // self.P

    @property
    def kchunks(self) -> int:       # 128-row chunks of the padded f_in
        return (self.f_in + self.P - 1) // self.P

    @property
    def f_pad(self) -> int:
        return self.kchunks * self.P

    @property
    def pad_gid(self) -> int:       # index of the all-zero table row
        return self.n_cores * self.npc

    def groups(self, kg: list[int]):
        """[(tile0, gsz, K)] for the conv gather groups."""
        out = []
        t = 0
        for K in kg:
            gsz = min(self.group, self.ntiles - t)
            out.append((t, gsz, K))
            t += gsz
        assert t == self.ntiles
        return out

    def chunks(self, kg: list[int]):
        """Split groups into runs whose gather descriptors fit one context."""
        runs, cur, cum = [], [], 0
        for item in self.groups(kg):
            _, gsz, K = item
            d = self.P * gsz * K
            assert d <= self.max_ctx_descs
            if cum + d > self.max_ctx_descs and cur:
                runs.append(cur)
                cur, cum = [], 0
            cur.append(item)
            cum += d
        if cur:
            runs.append(cur)
        return runs

    @property
    def n_groups(self) -> int:
        return (self.ntiles + self.group - 1) // self.group


FULL = Cfg()
NEG_BIG = -1.0e30


# --------------------------------------------------------------------------
# host-side preprocessing
# --------------------------------------------------------------------------

def preprocess(cfg: Cfg, x: np.ndarray, edge_index: np.ndarray):
    """Shard + layout transform. Returns (per_core list of dicts, kg, order_c)."""
    P, NPC, NPCR = cfg.P, cfg.npc, cfg.npc_raw
    n, C = cfg.n_nodes, cfg.n_cores

    src = edge_index[0].astype(np.int64)
    dst = edge_index[1].astype(np.int64)
    loop = np.arange(n, dtype=np.int64)
    src = np.concatenate([src, loop])
    dst = np.concatenate([dst, loop])

    core_of_dst = dst // NPCR

    # pass 1: per-core degree sort -> rank of each node within its core
    order_c, rank_c, deg_c = [], [], []
    for c in range(C):
        m = core_of_dst == c
        ld = dst[m] - c * NPCR
        deg = np.bincount(ld, minlength=NPCR)
        order = np.argsort(deg, kind="stable")      # ascending degree
        rank = np.empty(NPCR, np.int64)
        rank[order] = np.arange(NPCR)
        order_c.append(order)
        rank_c.append(rank)
        deg_c.append(deg)

    # new global id after per-core permutation + padding
    new_gid = np.empty(n, np.int64)
    for c in range(C):
        new_gid[c * NPCR:(c + 1) * NPCR] = c * NPC + rank_c[c]

    # per-group K (max in-degree in the group, shared across cores for SPMD)
    ngrp = cfg.n_groups
    kg = np.zeros(ngrp, np.int64)
    for c in range(C):
        degp = np.zeros(NPC, np.int64)
        degp[rank_c[c]] = deg_c[c]
        for g in range(ngrp):
            t0 = g * cfg.group
            gsz = min(cfg.group, cfg.ntiles - t0)
            kmax = degp[t0 * P:(t0 + gsz) * P].max()
            kg[g] = max(kg[g], kmax)
    kg = [int(max(k, 1)) for k in kg]
    kmax_all = max(kg)

    per_core = []
    for c in range(C):
        m = core_of_dst == c
        ld = dst[m] - c * NPCR
        gs = new_gid[src[m]]
        er = rank_c[c][ld]                       # dst rank of each edge
        eo = np.argsort(er, kind="stable")
        er_s = er[eo]
        gs_s = gs[eo]
        starts = np.zeros(NPC + 1, np.int64)
        np.cumsum(np.bincount(er_s, minlength=NPC), out=starts[1:])
        k_e = np.arange(er_s.size) - starts[er_s]
        M = np.full((NPC, kmax_all), cfg.pad_gid, np.int32)
        M[er_s, k_e] = gs_s.astype(np.int32)

        # idx layout: per group a [128, gsz*K] block, col = t_in_g*K + k
        blocks = []
        for (t0, gsz, K) in cfg.groups(kg):
            blk = M[t0 * P:(t0 + gsz) * P, :K]           # [gsz*128, K]
            blk = blk.reshape(gsz, P, K).transpose(1, 0, 2).reshape(P, gsz * K)
            blocks.append(blk)
        idx = np.ascontiguousarray(np.concatenate(blocks, axis=1))

        # x shard: permuted, padded, transposed, f-padded
        xs = x[c * NPCR:(c + 1) * NPCR][order_c[c]]      # [NPCR, f_in]
        xt = np.zeros((cfg.f_pad, NPC), np.float32)
        xt[:cfg.f_in, :NPCR] = xs.T
        per_core.append({"xt": np.ascontiguousarray(xt), "idx": idx})

    return per_core, kg, order_c


# --------------------------------------------------------------------------
# device kernel builder
# --------------------------------------------------------------------------

def build_kernel(cfg: Cfg, kg: list[int], phases: str = "ABCDE"):
    import concourse.bacc as bacc
    import concourse.tile as tile
    from concourse import bass, mybir
    from concourse.masks import make_identity

    P = cfg.P
    NH, NCO, ROW = cfg.nh, cfg.nc_out, cfg.row
    NPC, NT, KC = cfg.npc, cfg.ntiles, cfg.kchunks
    NFULL = cfg.n_cores * NPC
    f32 = mybir.dt.float32
    bf16 = mybir.dt.bfloat16
    i32 = mybir.dt.int32
    AX = mybir.AxisListType.X
    OP = mybir.AluOpType
    AF = mybir.ActivationFunctionType
    slot_cols = sum(gsz * K for (_, gsz, K) in cfg.groups(kg))

    nc = bacc.Bacc("TRN2", target_bir_lowering=False, debug=False,
                   num_devices=cfg.n_cores)

    xt_d = nc.dram_tensor("xt", [cfg.f_pad, NPC], f32, kind="ExternalInput")
    idx_d = nc.dram_tensor("idx", [P, slot_cols], i32, kind="ExternalInput")
    w1_d = nc.dram_tensor("w1p", [P, KC * NH], f32, kind="ExternalInput")
    b1_d = nc.dram_tensor("b1r", [P, NH], f32, kind="ExternalInput")
    w4_d = nc.dram_tensor("w4r", [NH, NCO], f32, kind="ExternalInput")
    b4_d = nc.dram_tensor("b4r", [P, NCO], f32, kind="ExternalInput")
    be_d = nc.dram_tensor("beta3r", [P, 1], f32, kind="ExternalInput")
    out_d = nc.dram_tensor("out", [NPC, NCO], f32, kind="ExternalOutput")

    tabA_l = nc.dram_tensor("tabA_l", [NPC, ROW], bf16)
    tabA_f = nc.dram_tensor("tabA_f", [NFULL + 2, ROW], bf16, addr_space="Shared")
    tabB_l = nc.dram_tensor("tabB_l", [NPC, ROW], bf16)
    tabB_f = nc.dram_tensor("tabB_f", [NFULL + 2, ROW], bf16, addr_space="Shared")

    # persistent SBUF (survives across TileContexts)
    def sb(name, shape, dtype):
        return nc.alloc_sbuf_tensor(name, list(shape), dtype)

    ident = sb("ident", [P, P], bf16)
    ident_f = sb("identf", [P, P], f32)
    zeros = sb("zeros", [P, P], f32)
    w1_sb = sb("w1sb", [P, KC * NH], f32)
    b1_sb = sb("b1sb", [P, NH], f32)
    w4_sb = sb("w4sb", [NH, NCO], f32)
    b4_sb = sb("b4sb", [P, NCO], f32)
    be_sb = sb("besb", [P, 1], f32)
    h_sb = sb("hsb", [P, NT * NH], f32)
    rows_sb = sb("rowssb", [P, NT * ROW], bf16)
    sq_sb = sb("sqsb", [P, NT * NH], f32)
    ss_sb = sb("sssb", [P, NT], f32)
    inv_sb = sb("invsb", [P, NT], f32)

    def epilogue_rows(tab_local):
        """h_sb -> inv norm -> rows_sb -> DMA to tab_local."""
        h3v = h_sb.ap().rearrange("p (t j) -> p t j", t=NT)
        nc.vector.tensor_mul(sq_sb.ap(), h_sb.ap(), h_sb.ap())
        nc.vector.reduce_sum(
            ss_sb.ap(), sq_sb.ap().rearrange("p (t j) -> p t j", t=NT), axis=AX)
        nc.scalar.sqrt(ss_sb.ap(), ss_sb.ap())
        nc.vector.tensor_scalar_add(ss_sb.ap(), ss_sb.ap(), 1.0e-12)
        nc.vector.reciprocal(inv_sb.ap(), ss_sb.ap())
        rv = rows_sb.ap().rearrange("p (t j) -> p t j", t=NT)
        nc.vector.tensor_copy(rv[:, :, 0:16], h3v)
        nc.vector.tensor_copy(rv[:, :, 16], inv_sb.ap())
        nc.vector.tensor_copy(rv[:, :, 17], zeros.ap()[:, 0:NT])
        nc.sync.dma_start(
            out=tab_local[:, :].rearrange("(t p) j -> p t j", p=P),
            in_=rv)

    def allgather(tab_local, tab_full):
        nc.gpsimd.collective_compute(
            "AllGather", OP.bypass,
            replica_groups=[list(range(cfg.n_cores))],
            ins=[tab_local.ap().opt()],
            outs=[tab_full.ap()[0:NFULL, :].opt()])

    # ---------------- phase A: consts, L1, table A, AG1 --------------------
    with tile.TileContext(nc) as tc:
        make_identity(nc, ident.ap())
        make_identity(nc, ident_f.ap())
        nc.gpsimd.memset(zeros.ap(), 0.0)
        nc.gpsimd.memset(rows_sb.ap(), 0.0)
        nc.sync.dma_start(out=w1_sb.ap(), in_=w1_d[:, :])
        nc.sync.dma_start(out=b1_sb.ap(), in_=b1_d[:, :])
        nc.sync.dma_start(out=w4_sb.ap(), in_=w4_d[:, :])
        nc.sync.dma_start(out=b4_sb.ap(), in_=b4_d[:, :])
        nc.sync.dma_start(out=be_sb.ap(), in_=be_d[:, :])
        with ExitStack() as ctx:
            const = ctx.enter_context(tc.tile_pool(name="pad", bufs=1))
            padrow = const.tile([1, ROW], bf16, tag="padrow")
            nc.gpsimd.memset(padrow[:], 0.0)
            nc.gpsimd.memset(padrow[:1, 17:18], NEG_BIG)
            nc.sync.dma_start(
                out=tabA_f[NFULL:NFULL + 2, :][None, :, :],
                in_=padrow[:1, None, :].to_broadcast([1, 2, ROW]))
            nc.sync.dma_start(
                out=tabB_f[NFULL:NFULL + 2, :][None, :, :],
                in_=padrow[:1, None, :].to_broadcast([1, 2, ROW]))

        with tc.tile_pool(name="l1x", bufs=3) as xp, \
             tc.tile_pool(name="l1p", bufs=4, space="PSUM") as pp:
            for t in range(NT):
                xw = xp.tile([P, KC * P], f32, tag="xw")
                src = xt_d[:, :].rearrange("(c p) m -> p c m", p=P)[:, :, t * P:(t + 1) * P]
                nc.sync.dma_start(
                    out=xw[:].rearrange("p (c j) -> p c j", c=KC), in_=src)
                ps = pp.tile([P, NH], f32, tag="l1ps")
                for c in range(KC):
                    nc.tensor.matmul(
                        out=ps[:], lhsT=xw[:, c * P:(c + 1) * P],
                        rhs=w1_sb.ap()[:, c * NH:(c + 1) * NH],
                        start=(c == 0), stop=(c == KC - 1))
                hsl = h_sb.ap()[:, t * NH:(t + 1) * NH]
                nc.vector.tensor_add(hsl, ps[:], b1_sb.ap())
                nc.vector.tensor_scalar_max(hsl, hsl, 0.0)
        epilogue_rows(tabA_l)
        allgather(tabA_l, tabA_f)

    # ---------------- conv layer (one TileContext per chunk) ---------------
    def conv(tab_local, tab_full, beta_ap_fn):
        off = 0
        t_seen = 0
        for run in cfg.chunks(kg):
            with tile.TileContext(nc) as tc:
                with tc.tile_pool(name="cv", bufs=3) as cv, \
                     tc.tile_pool(name="cvp", bufs=2, space="PSUM") as cvp:
                    for (t0, gsz, K) in run:
                        gk = gsz * K
                        idx_sb = cv.tile([P, gk], i32, tag="idx")
                        nc.sync.dma_start(
                            out=idx_sb[:], in_=idx_d[:, off:off + gk])
                        hs = cv.tile([P, gk * ROW], bf16, tag="hs")
                        # HW indirect DMA = ONE index per partition reading
                        # contiguous elems; one gather per slot column. The
                        # completion sem fires at descriptor-generation, so a
                        # trailing plain SWDGE DMA on the same ring provides a
                        # data-landed fence for the whole group.
                        for j in range(gk):
                            nc.gpsimd.indirect_dma_start(
                                out=hs[:, j * ROW:(j + 1) * ROW],
                                out_offset=None,
                                in_=tab_full.ap(),
                                in_offset=bass.IndirectOffsetOnAxis(
                                    ap=idx_sb[:, j:j + 1], axis=0),
                            )
                        guard = cv.tile([P, 4], i32, tag="guard")
                        flush = nc.gpsimd.dma_start(
                            out=guard[:], in_=idx_d[:, 0:4])
                        hd = cv.tile([P, gsz * ROW], bf16, tag="hd")
                        nc.sync.dma_start(
                            out=hd[:].rearrange("p (g j) -> p g j", g=gsz),
                            in_=tab_local[t0 * P:(t0 + gsz) * P, :].rearrange(
                                "(g p) j -> p g j", p=P))

                        hs4 = hs[:].rearrange("p (g k j) -> p g k j", g=gsz, k=K)
                        hd3 = hd[:].rearrange("p (g j) -> p g j", g=gsz)
                        tmp = cv.tile([P, gk * NH], bf16, tag="tmp")
                        tm4 = tmp[:].rearrange("p (g k j) -> p g k j", g=gsz, k=K)
                        mul1 = nc.vector.tensor_mul(
                            tm4, hs4[:, :, :, 0:16],
                            hd3[:, :, None, 0:16].to_broadcast([P, gsz, K, 16]))
                        bass._add_dep_helper(
                            mul1.ins, flush.ins, sync=True,
                            reason="hs consumer waits for gather ring drain")
                        alpha = cv.tile([P, gk], f32, tag="alpha")
                        al3 = alpha[:].rearrange("p (g k) -> p g k", g=gsz)
                        nc.vector.reduce_sum(
                            alpha[:],
                            tmp[:].rearrange("p (gk j) -> p gk j", j=NH), axis=AX)
                        nc.vector.tensor_mul(al3, al3, hs4[:, :, :, 16])
                        invd = cv.tile([P, gsz], f32, tag="invd")
                        beta_ap = beta_ap_fn()
                        if beta_ap is None:
                            nc.vector.tensor_copy(invd[:], hd3[:, :, 16])
                        else:
                            nc.vector.tensor_scalar_mul(
                                invd[:], hd3[:, :, 16], beta_ap)
                        nc.vector.tensor_mul(
                            al3, al3,
                            invd[:][:, :, None].to_broadcast([P, gsz, K]))
                        nc.vector.tensor_add(al3, al3, hs4[:, :, :, 17])
                        nm = cv.tile([P, gsz], f32, tag="nm")
                        nc.vector.reduce_max(nm[:], al3, axis=AX, negate=True)
                        nc.vector.tensor_add(
                            al3, al3,
                            nm[:][:, :, None].to_broadcast([P, gsz, K]))
                        e_bf = cv.tile([P, gk], bf16, tag="e")
                        nc.scalar.activation(e_bf[:], alpha[:], AF.Exp)
                        s = cv.tile([P, gsz], f32, tag="s")
                        nc.vector.reduce_sum(
                            s[:], e_bf[:].rearrange("p (g k) -> p g k", g=gsz),
                            axis=AX)
                        nc.vector.tensor_scalar_add(s[:], s[:], 1.0e-16)
                        r = cv.tile([P, gsz], f32, tag="r")
                        nc.vector.reciprocal(r[:], s[:])
                        coef = cv.tile([P, gk], bf16, tag="coef")
                        nc.vector.tensor_mul(
                            coef[:].rearrange("p (g k) -> p g k", g=gsz),
                            e_bf[:].rearrange("p (g k) -> p g k", g=gsz),
                            r[:][:, :, None].to_broadcast([P, gsz, K]))
                        tmp2 = cv.tile([P, gk * NH], bf16, tag="tmp2")
                        t24 = tmp2[:].rearrange("p (g k j) -> p g k j", g=gsz, k=K)
                        nc.vector.tensor_mul(
                            t24, hs4[:, :, :, 0:16],
                            coef[:].rearrange("p (g k) -> p g k", g=gsz)
                            [:, :, :, None].to_broadcast([P, gsz, K, 16]))
                        h2v = h_sb.ap()[:, t0 * NH:(t0 + gsz) * NH]
                        nc.vector.reduce_sum(
                            h2v,
                            tmp2[:].rearrange(
                                "p (g k j) -> p g j k", g=gsz, k=K),
                            axis=AX)
                        off += gk
                        t_seen += gsz
        assert t_seen == NT

    if "B" in phases:
        conv(tabA_l, tabA_f, lambda: None)

    # ---------------- phase C: table B + AG2 -------------------------------
    if "C" in phases:
        with tile.TileContext(nc) as tc:
            epilogue_rows(tabB_l)
            allgather(tabB_l, tabB_f)

    if "D" in phases:
        conv(tabB_l, tabB_f, lambda: be_sb.ap()[:, 0:1])

    # ---------------- head + log_softmax -----------------------------------
    if "G" in phases:
        # debug: gather group 0 from tabA_f and dump raw rows (as f32)
        (t0g, gszg, Kg) = cfg.groups(kg)[0]
        gkg = gszg * Kg
        with tile.TileContext(nc) as tc:
            with tc.tile_pool(name="dbg", bufs=1) as dbg:
                idx_sb = dbg.tile([P, gkg], i32, tag="idx")
                nc.sync.dma_start(out=idx_sb[:], in_=idx_d[:, 0:gkg])
                hs = dbg.tile([P, gkg * ROW], bf16, tag="hs")
                for j in range(gkg):
                    nc.gpsimd.indirect_dma_start(
                        out=hs[:, j * ROW:(j + 1) * ROW], out_offset=None,
                        in_=tabA_f.ap(),
                        in_offset=bass.IndirectOffsetOnAxis(
                            ap=idx_sb[:, j:j + 1], axis=0))
                guard = dbg.tile([P, 4], i32, tag="guard")
                flush = nc.gpsimd.dma_start(out=guard[:], in_=idx_d[:, 0:4])
                ncols = min(gkg * ROW, (NPC // P) * NCO * (NPC // P and 1) * 980)
                ncols = min(gkg * ROW, 980)
                hf = dbg.tile([P, ncols], f32, tag="hf")
                cp = nc.vector.tensor_copy(hf[:], hs[:, 0:ncols])
                bass._add_dep_helper(
                    cp.ins, flush.ins, sync=True, reason="debug drain")
                ov = out_d.ap().rearrange("(p q) j -> p (q j)", p=P)
                nc.sync.dma_start(out=ov[:, 0:ncols], in_=hf[:])
        nc.compile()
        return nc

    if "E" not in phases:
        # debug: dump h_sb (and inv_sb) into out
        with tile.TileContext(nc) as tc:
            ov = out_d.ap().rearrange("(p q) j -> p (q j)", p=P)
            nc.sync.dma_start(out=ov[:, 0:NT * NH], in_=h_sb.ap())
            nc.sync.dma_start(out=ov[:, NT * NH:NT * NH + NT], in_=inv_sb.ap())
        nc.compile()
        return nc

    with tile.TileContext(nc) as tc:
        with tc.tile_pool(name="hd", bufs=1) as hp, \
             tc.tile_pool(name="hdp", bufs=4, space="PSUM") as hpp:
            h3t = hp.tile([NH, NT * P], f32, tag="h3t")
            for t in range(NT):
                pst = hpp.tile([NH, P], f32, tag="pst")
                nc.tensor.transpose(
                    out=pst[:], in_=h_sb.ap()[:, t * NH:(t + 1) * NH],
                    identity=ident_f.ap())
                nc.vector.tensor_copy(h3t[:, t * P:(t + 1) * P], pst[:])
            lg = hp.tile([P, NT * NCO], f32, tag="lg")
            for t in range(NT):
                psl = hpp.tile([P, NCO], f32, tag="psl")
                nc.tensor.matmul(
                    out=psl[:], lhsT=h3t[:, t * P:(t + 1) * P], rhs=w4_sb.ap(),
                    start=True, stop=True)
                nc.vector.tensor_add(
                    lg[:, t * NCO:(t + 1) * NCO], psl[:], b4_sb.ap())
            lg3 = lg[:].rearrange("p (t j) -> p t j", t=NT)
            nm = hp.tile([P, NT], f32, tag="hnm")
            nc.vector.reduce_max(nm[:], lg3, axis=AX, negate=True)
            nc.vector.tensor_add(
                lg3, lg3, nm[:][:, :, None].to_broadcast([P, NT, NCO]))
            ex = hp.tile([P, NT * NCO], f32, tag="ex")
            nc.scalar.activation(ex[:], lg[:], AF.Exp)
            s = hp.tile([P, NT], f32, tag="hs_sum")
            nc.vector.reduce_sum(
                s[:], ex[:].rearrange("p (t j) -> p t j", t=NT), axis=AX)
            ls = hp.tile([P, NT], f32, tag="ls")
            nc.scalar.activation(ls[:], s[:], AF.Ln)
            nc.vector.tensor_sub(
                lg3, lg3, ls[:][:, :, None].to_broadcast([P, NT, NCO]))
            nc.sync.dma_start(
                out=out_d[:, :].rearrange("(t p) j -> p t j", p=P),
                in_=lg3)

    nc.compile()
    return nc


# --------------------------------------------------------------------------
# entry point
# --------------------------------------------------------------------------

def run(cfg: Cfg, inputs: dict, trace: bool = False):
    from concourse import bass_utils

    x = np.asarray(inputs["x"], np.float32)
    edge_index = np.asarray(inputs["edge_index"])
    W1 = np.asarray(inputs["W1"], np.float32)
    b1 = np.asarray(inputs["b1"], np.float32)
    W4 = np.asarray(inputs["W4"], np.float32)
    b4 = np.asarray(inputs["b4"], np.float32)
    beta3 = np.asarray(inputs["beta3"], np.float32)

    per_core, kg, order_c = preprocess(cfg, x, edge_index)
    nc = build_kernel(cfg, kg)

    P, KC, NH = cfg.P, cfg.kchunks, cfg.nh
    w1p = np.zeros((cfg.f_pad, NH), np.float32)
    w1p[:cfg.f_in] = W1
    w1p = np.ascontiguousarray(
        w1p.reshape(KC, P, NH).transpose(1, 0, 2).reshape(P, KC * NH))
    b1r = np.ascontiguousarray(np.broadcast_to(b1[None, :], (P, NH)))
    b4r = np.ascontiguousarray(np.broadcast_to(b4[None, :], (P, cfg.nc_out)))
    ber = np.ascontiguousarray(np.broadcast_to(beta3[None, :], (P, 1)))

    in_maps = []
    for c in range(cfg.n_cores):
        in_maps.append({
            "xt": per_core[c]["xt"],
            "idx": per_core[c]["idx"],
            "w1p": w1p, "b1r": b1r, "w4r": np.ascontiguousarray(W4),
            "b4r": b4r, "beta3r": ber,
        })

    res = bass_utils.run_bass_kernel_spmd(
        nc, in_maps, core_ids=list(range(cfg.n_cores)), trace=trace)

    out = np.empty((cfg.n_nodes, cfg.nc_out), np.float32)
    for c in range(cfg.n_cores):
        oc = np.asarray(res.results[c]["out"])[:cfg.npc_raw]
        out[c * cfg.npc_raw + order_c[c]] = oc
    return out, res


def kernel(**inputs) -> np.ndarray:
    out, _ = run(FULL, inputs, trace=False)
    return out



# revision 9
# speedup vs baseline: 1.0311x; 1.0279x over previous
"""AGNNet (2-layer AGNN conv + linear head) distributed over 8 trn2 NeuronCores.

Strategy (graph/data parallel, per sharding hint):
  - nodes sharded by dst range: core c owns nodes [c*6250, (c+1)*6250)
  - host groups edges by dst, degree-sorts each core's local nodes (undone on
    output), pads each 128-node tile's in-edge lists to a per-group slot count
  - device: L1 matmul from host-transposed x shard; build a bf16 row table
    [h(16), inv_norm, bias, pad] per node; AllGather the table; per node-tile
    indirect-DMA gather of neighbor rows + DVE/ACT softmax; PE accumulates the
    weighted sum over slots; second conv identical; head matmul + log_softmax.
  - a dedicated all-zero table row (bias column = -1e30) backs padding slots so
    they vanish in the softmax and contribute 0 to the weighted sum.
  - the kernel is split into sequential TileContexts: walrus tracks SWDGE
    (indirect DMA) ring occupancy in a cumulative 16-bit semaphore value, so
    each context must stay under ~60k gather descriptors; the context-exit
    drain+sem-clear resets the counter.
"""

import math
from contextlib import ExitStack
from dataclasses import dataclass

import numpy as np


@dataclass
class Cfg:
    n_cores: int = 8
    n_nodes: int = 50000
    f_in: int = 2000
    nh: int = 16
    nc_out: int = 20
    group: int = 4          # node tiles per conv gather group
    row: int = 20           # table row elems: 16 h, [16]=invn, [17]=bias, 18-19 pad
    P: int = 128
    max_ctx_descs: int = 55000   # SWDGE descriptor budget per TileContext

    @property
    def npc_raw(self) -> int:       # real nodes per core
        return self.n_nodes // self.n_cores

    @property
    def npc(self) -> int:           # padded nodes per core (mult of 128)
        return ((self.npc_raw + self.P - 1) // self.P) * self.P

    @property
    def ntiles(self) -> int:
        return self.npc // self.P

    @property
    def kchunks(self) -> int:       # 128-row chunks of the padded f_in
        return (self.f_in + self.P - 1) // self.P

    @property
    def f_pad(self) -> int:
        return self.kchunks * self.P

    @property
    def pad_gid(self) -> int:       # index of the all-zero table row
        return self.n_cores * self.npc

    def groups(self, kg: list[int]):
        """[(tile0, gsz, K)] for the conv gather groups."""
        out = []
        t = 0
        for K in kg:
            gsz = min(self.group, self.ntiles - t)
            out.append((t, gsz, K))
            t += gsz
        assert t == self.ntiles
        return out

    def chunks(self, kg: list[int]):
        """Split groups into runs whose gather descriptors fit one context."""
        runs, cur, cum = [], [], 0
        for item in self.groups(kg):
            _, gsz, K = item
            d = self.P * gsz * K
            assert d <= self.max_ctx_descs
            if cum + d > self.max_ctx_descs and cur:
                runs.append(cur)
                cur, cum = [], 0
            cur.append(item)
            cum += d
        if cur:
            runs.append(cur)
        return runs

    @property
    def n_groups(self) -> int:
        return (self.ntiles + self.group - 1) // self.group


FULL = Cfg()
NEG_BIG = -1.0e30


# --------------------------------------------------------------------------
# host-side preprocessing
# --------------------------------------------------------------------------

def preprocess(cfg: Cfg, x: np.ndarray, edge_index: np.ndarray):
    """Shard + layout transform. Returns (per_core list of dicts, kg, order_c)."""
    P, NPC, NPCR = cfg.P, cfg.npc, cfg.npc_raw
    n, C = cfg.n_nodes, cfg.n_cores

    src = edge_index[0].astype(np.int64)
    dst = edge_index[1].astype(np.int64)
    loop = np.arange(n, dtype=np.int64)
    src = np.concatenate([src, loop])
    dst = np.concatenate([dst, loop])

    core_of_dst = dst // NPCR

    # pass 1: per-core degree sort -> rank of each node within its core
    order_c, rank_c, deg_c = [], [], []
    for c in range(C):
        m = core_of_dst == c
        ld = dst[m] - c * NPCR
        deg = np.bincount(ld, minlength=NPCR)
        order = np.argsort(deg, kind="stable")      # ascending degree
        rank = np.empty(NPCR, np.int64)
        rank[order] = np.arange(NPCR)
        order_c.append(order)
        rank_c.append(rank)
        deg_c.append(deg)

    # new global id after per-core permutation + padding
    new_gid = np.empty(n, np.int64)
    for c in range(C):
        new_gid[c * NPCR:(c + 1) * NPCR] = c * NPC + rank_c[c]

    # per-group K (max in-degree in the group, shared across cores for SPMD)
    ngrp = cfg.n_groups
    kg = np.zeros(ngrp, np.int64)
    for c in range(C):
        degp = np.zeros(NPC, np.int64)
        degp[rank_c[c]] = deg_c[c]
        for g in range(ngrp):
            t0 = g * cfg.group
            gsz = min(cfg.group, cfg.ntiles - t0)
            kmax = degp[t0 * P:(t0 + gsz) * P].max()
            kg[g] = max(kg[g], kmax)
    kg = [int(max(k, 1)) for k in kg]
    kmax_all = max(kg)

    per_core = []
    for c in range(C):
        m = core_of_dst == c
        ld = dst[m] - c * NPCR
        gs = new_gid[src[m]]
        er = rank_c[c][ld]                       # dst rank of each edge
        eo = np.argsort(er, kind="stable")
        er_s = er[eo]
        gs_s = gs[eo]
        starts = np.zeros(NPC + 1, np.int64)
        np.cumsum(np.bincount(er_s, minlength=NPC), out=starts[1:])
        k_e = np.arange(er_s.size) - starts[er_s]
        M = np.full((NPC, kmax_all), cfg.pad_gid, np.int32)
        M[er_s, k_e] = gs_s.astype(np.int32)

        # idx layout: per group a [128, gsz*K] block, col = t_in_g*K + k
        blocks = []
        for (t0, gsz, K) in cfg.groups(kg):
            blk = M[t0 * P:(t0 + gsz) * P, :K]           # [gsz*128, K]
            blk = blk.reshape(gsz, P, K).transpose(1, 0, 2).reshape(P, gsz * K)
            blocks.append(blk)
        idx = np.ascontiguousarray(np.concatenate(blocks, axis=1))

        # x shard: permuted, padded, transposed, f-padded
        xs = x[c * NPCR:(c + 1) * NPCR][order_c[c]]      # [NPCR, f_in]
        xt = np.zeros((cfg.f_pad, NPC), np.float32)
        xt[:cfg.f_in, :NPCR] = xs.T
        per_core.append({"xt": np.ascontiguousarray(xt), "idx": idx})

    return per_core, kg, order_c


# --------------------------------------------------------------------------
# device kernel builder
# --------------------------------------------------------------------------

def build_kernel(cfg: Cfg, kg: list[int], phases: str = "ABCDE"):
    import concourse.bacc as bacc
    import concourse.tile as tile
    from concourse import bass, mybir
    from concourse.masks import make_identity

    P = cfg.P
    NH, NCO, ROW = cfg.nh, cfg.nc_out, cfg.row
    NPC, NT, KC = cfg.npc, cfg.ntiles, cfg.kchunks
    NFULL = cfg.n_cores * NPC
    f32 = mybir.dt.float32
    bf16 = mybir.dt.bfloat16
    i32 = mybir.dt.int32
    AX = mybir.AxisListType.X
    OP = mybir.AluOpType
    AF = mybir.ActivationFunctionType
    slot_cols = sum(gsz * K for (_, gsz, K) in cfg.groups(kg))

    nc = bacc.Bacc("TRN2", target_bir_lowering=False, debug=False,
                   num_devices=cfg.n_cores)

    xt_d = nc.dram_tensor("xt", [cfg.f_pad, NPC], bf16, kind="ExternalInput")
    idx_d = nc.dram_tensor("idx", [P, slot_cols], i32, kind="ExternalInput")
    w1_d = nc.dram_tensor("w1p", [P, KC * NH], bf16, kind="ExternalInput")
    b1_d = nc.dram_tensor("b1r", [P, NH], f32, kind="ExternalInput")
    w4_d = nc.dram_tensor("w4r", [NH, NCO], f32, kind="ExternalInput")
    b4_d = nc.dram_tensor("b4r", [P, NCO], f32, kind="ExternalInput")
    be_d = nc.dram_tensor("beta3r", [P, 1], f32, kind="ExternalInput")
    out_d = nc.dram_tensor("out", [NPC, NCO], f32, kind="ExternalOutput")

    tabA_l = nc.dram_tensor("tabA_l", [NPC, ROW], bf16)
    tabA_f = nc.dram_tensor("tabA_f", [NFULL + 2, ROW], bf16, addr_space="Shared")
    tabB_l = nc.dram_tensor("tabB_l", [NPC, ROW], bf16)
    tabB_f = nc.dram_tensor("tabB_f", [NFULL + 2, ROW], bf16, addr_space="Shared")

    # persistent SBUF (survives across TileContexts)
    def sb(name, shape, dtype):
        return nc.alloc_sbuf_tensor(name, list(shape), dtype)

    ident = sb("ident", [P, P], bf16)
    ident_f = sb("identf", [P, P], f32)
    zeros = sb("zeros", [P, P], f32)
    w1_sb = sb("w1sb", [P, KC * NH], bf16)
    b1_sb = sb("b1sb", [P, NH], f32)
    w4_sb = sb("w4sb", [NH, NCO], f32)
    b4_sb = sb("b4sb", [P, NCO], f32)
    be_sb = sb("besb", [P, 1], f32)
    h_sb = sb("hsb", [P, NT * NH], f32)
    rows_sb = sb("rowssb", [P, NT * ROW], bf16)
    sq_sb = sb("sqsb", [P, NT * NH], f32)
    ss_sb = sb("sssb", [P, NT], f32)
    inv_sb = sb("invsb", [P, NT], f32)

    def epilogue_rows(tab_local):
        """h_sb -> inv norm -> rows_sb -> DMA to tab_local."""
        h3v = h_sb.ap().rearrange("p (t j) -> p t j", t=NT)
        nc.vector.tensor_mul(sq_sb.ap(), h_sb.ap(), h_sb.ap())
        nc.vector.reduce_sum(
            ss_sb.ap(), sq_sb.ap().rearrange("p (t j) -> p t j", t=NT), axis=AX)
        nc.scalar.sqrt(ss_sb.ap(), ss_sb.ap())
        nc.vector.tensor_scalar_add(ss_sb.ap(), ss_sb.ap(), 1.0e-12)
        nc.vector.reciprocal(inv_sb.ap(), ss_sb.ap())
        rv = rows_sb.ap().rearrange("p (t j) -> p t j", t=NT)
        nc.vector.tensor_copy(rv[:, :, 0:16], h3v)
        nc.vector.tensor_copy(rv[:, :, 16], inv_sb.ap())
        nc.vector.tensor_copy(rv[:, :, 17], zeros.ap()[:, 0:NT])
        nc.sync.dma_start(
            out=tab_local[:, :].rearrange("(t p) j -> p t j", p=P),
            in_=rv)

    def allgather(tab_local, tab_full):
        nc.gpsimd.collective_compute(
            "AllGather", OP.bypass,
            replica_groups=[list(range(cfg.n_cores))],
            ins=[tab_local.ap().opt()],
            outs=[tab_full.ap()[0:NFULL, :].opt()])

    # ---------------- phase A: consts, L1, table A, AG1 --------------------
    with tile.TileContext(nc) as tc:
        make_identity(nc, ident.ap())
        make_identity(nc, ident_f.ap())
        nc.gpsimd.memset(zeros.ap(), 0.0)
        nc.gpsimd.memset(rows_sb.ap(), 0.0)
        nc.sync.dma_start(out=w1_sb.ap(), in_=w1_d[:, :])
        nc.sync.dma_start(out=b1_sb.ap(), in_=b1_d[:, :])
        nc.sync.dma_start(out=w4_sb.ap(), in_=w4_d[:, :])
        nc.sync.dma_start(out=b4_sb.ap(), in_=b4_d[:, :])
        nc.sync.dma_start(out=be_sb.ap(), in_=be_d[:, :])
        with ExitStack() as ctx:
            const = ctx.enter_context(tc.tile_pool(name="pad", bufs=1))
            padrow = const.tile([1, ROW], bf16, tag="padrow")
            nc.gpsimd.memset(padrow[:], 0.0)
            nc.gpsimd.memset(padrow[:1, 17:18], NEG_BIG)
            nc.sync.dma_start(
                out=tabA_f[NFULL:NFULL + 2, :][None, :, :],
                in_=padrow[:1, None, :].to_broadcast([1, 2, ROW]))
            nc.sync.dma_start(
                out=tabB_f[NFULL:NFULL + 2, :][None, :, :],
                in_=padrow[:1, None, :].to_broadcast([1, 2, ROW]))

        with tc.tile_pool(name="l1x", bufs=3) as xp, \
             tc.tile_pool(name="l1p", bufs=4, space="PSUM") as pp:
            for t in range(NT):
                xw = xp.tile([P, KC * P], bf16, tag="xw")
                src = xt_d[:, :].rearrange("(c p) m -> p c m", p=P)[:, :, t * P:(t + 1) * P]
                nc.sync.dma_start(
                    out=xw[:].rearrange("p (c j) -> p c j", c=KC), in_=src)
                ps = pp.tile([P, NH], f32, tag="l1ps")
                for c in range(KC):
                    nc.tensor.matmul(
                        out=ps[:], lhsT=xw[:, c * P:(c + 1) * P],
                        rhs=w1_sb.ap()[:, c * NH:(c + 1) * NH],
                        start=(c == 0), stop=(c == KC - 1))
                hsl = h_sb.ap()[:, t * NH:(t + 1) * NH]
                nc.vector.tensor_add(hsl, ps[:], b1_sb.ap())
                nc.vector.tensor_scalar_max(hsl, hsl, 0.0)
        epilogue_rows(tabA_l)
        allgather(tabA_l, tabA_f)

    # ---------------- conv layer (one TileContext per chunk) ---------------
    def conv(tab_local, tab_full, beta_ap_fn):
        off = 0
        t_seen = 0
        for run in cfg.chunks(kg):
            with tile.TileContext(nc) as tc:
                with tc.tile_pool(name="cv", bufs=3) as cv, \
                     tc.tile_pool(name="cvp", bufs=2, space="PSUM") as cvp:
                    for (t0, gsz, K) in run:
                        gk = gsz * K
                        idx_sb = cv.tile([P, gk], i32, tag="idx")
                        nc.sync.dma_start(
                            out=idx_sb[:], in_=idx_d[:, off:off + gk])
                        hs = cv.tile([P, gk * ROW], bf16, tag="hs")
                        # HW indirect DMA = ONE index per partition reading
                        # contiguous elems; one gather per slot column. The
                        # completion sem fires at descriptor-generation, so a
                        # trailing plain SWDGE DMA on the same ring provides a
                        # data-landed fence for the whole group.
                        for j in range(gk):
                            nc.gpsimd.indirect_dma_start(
                                out=hs[:, j * ROW:(j + 1) * ROW],
                                out_offset=None,
                                in_=tab_full.ap(),
                                in_offset=bass.IndirectOffsetOnAxis(
                                    ap=idx_sb[:, j:j + 1], axis=0),
                            )
                        guard = cv.tile([P, 4], i32, tag="guard")
                        flush = nc.gpsimd.dma_start(
                            out=guard[:], in_=idx_d[:, 0:4])
                        hd = cv.tile([P, gsz * ROW], bf16, tag="hd")
                        nc.sync.dma_start(
                            out=hd[:].rearrange("p (g j) -> p g j", g=gsz),
                            in_=tab_local[t0 * P:(t0 + gsz) * P, :].rearrange(
                                "(g p) j -> p g j", p=P))

                        hs4 = hs[:].rearrange("p (g k j) -> p g k j", g=gsz, k=K)
                        hd3 = hd[:].rearrange("p (g j) -> p g j", g=gsz)
                        tmp = cv.tile([P, gk * NH], bf16, tag="tmp")
                        tm4 = tmp[:].rearrange("p (g k j) -> p g k j", g=gsz, k=K)
                        mul1 = nc.vector.tensor_mul(
                            tm4, hs4[:, :, :, 0:16],
                            hd3[:, :, None, 0:16].to_broadcast([P, gsz, K, 16]))
                        bass._add_dep_helper(
                            mul1.ins, flush.ins, sync=True,
                            reason="hs consumer waits for gather ring drain")
                        alpha = cv.tile([P, gk], f32, tag="alpha")
                        al3 = alpha[:].rearrange("p (g k) -> p g k", g=gsz)
                        nc.vector.reduce_sum(
                            alpha[:],
                            tmp[:].rearrange("p (gk j) -> p gk j", j=NH), axis=AX)
                        nc.vector.tensor_mul(al3, al3, hs4[:, :, :, 16])
                        invd = cv.tile([P, gsz], f32, tag="invd")
                        beta_ap = beta_ap_fn()
                        if beta_ap is None:
                            nc.vector.tensor_copy(invd[:], hd3[:, :, 16])
                        else:
                            nc.vector.tensor_scalar_mul(
                                invd[:], hd3[:, :, 16], beta_ap)
                        nc.vector.tensor_mul(
                            al3, al3,
                            invd[:][:, :, None].to_broadcast([P, gsz, K]))
                        nc.vector.tensor_add(al3, al3, hs4[:, :, :, 17])
                        nm = cv.tile([P, gsz], f32, tag="nm")
                        nc.vector.reduce_max(nm[:], al3, axis=AX, negate=True)
                        nc.vector.tensor_add(
                            al3, al3,
                            nm[:][:, :, None].to_broadcast([P, gsz, K]))
                        e_bf = cv.tile([P, gk], bf16, tag="e")
                        nc.scalar.activation(e_bf[:], alpha[:], AF.Exp)
                        s = cv.tile([P, gsz], f32, tag="s")
                        nc.vector.reduce_sum(
                            s[:], e_bf[:].rearrange("p (g k) -> p g k", g=gsz),
                            axis=AX)
                        nc.vector.tensor_scalar_add(s[:], s[:], 1.0e-16)
                        r = cv.tile([P, gsz], f32, tag="r")
                        nc.vector.reciprocal(r[:], s[:])
                        coef = cv.tile([P, gk], bf16, tag="coef")
                        nc.vector.tensor_mul(
                            coef[:].rearrange("p (g k) -> p g k", g=gsz),
                            e_bf[:].rearrange("p (g k) -> p g k", g=gsz),
                            r[:][:, :, None].to_broadcast([P, gsz, K]))
                        tmp2 = cv.tile([P, gk * NH], bf16, tag="tmp2")
                        t24 = tmp2[:].rearrange("p (g k j) -> p g k j", g=gsz, k=K)
                        nc.vector.tensor_mul(
                            t24, hs4[:, :, :, 0:16],
                            coef[:].rearrange("p (g k) -> p g k", g=gsz)
                            [:, :, :, None].to_broadcast([P, gsz, K, 16]))
                        h2v = h_sb.ap()[:, t0 * NH:(t0 + gsz) * NH]
                        nc.vector.reduce_sum(
                            h2v,
                            tmp2[:].rearrange(
                                "p (g k j) -> p g j k", g=gsz, k=K),
                            axis=AX)
                        off += gk
                        t_seen += gsz
        assert t_seen == NT

    if "B" in phases:
        conv(tabA_l, tabA_f, lambda: None)

    # ---------------- phase C: table B + AG2 -------------------------------
    if "C" in phases:
        with tile.TileContext(nc) as tc:
            epilogue_rows(tabB_l)
            allgather(tabB_l, tabB_f)

    if "D" in phases:
        conv(tabB_l, tabB_f, lambda: be_sb.ap()[:, 0:1])

    # ---------------- head + log_softmax -----------------------------------
    if "G" in phases:
        # debug: gather group 0 from tabA_f and dump raw rows (as f32)
        (t0g, gszg, Kg) = cfg.groups(kg)[0]
        gkg = gszg * Kg
        with tile.TileContext(nc) as tc:
            with tc.tile_pool(name="dbg", bufs=1) as dbg:
                idx_sb = dbg.tile([P, gkg], i32, tag="idx")
                nc.sync.dma_start(out=idx_sb[:], in_=idx_d[:, 0:gkg])
                hs = dbg.tile([P, gkg * ROW], bf16, tag="hs")
                for j in range(gkg):
                    nc.gpsimd.indirect_dma_start(
                        out=hs[:, j * ROW:(j + 1) * ROW], out_offset=None,
                        in_=tabA_f.ap(),
                        in_offset=bass.IndirectOffsetOnAxis(
                            ap=idx_sb[:, j:j + 1], axis=0))
                guard = dbg.tile([P, 4], i32, tag="guard")
                flush = nc.gpsimd.dma_start(out=guard[:], in_=idx_d[:, 0:4])
                ncols = min(gkg * ROW, (NPC // P) * NCO * (NPC // P and 1) * 980)
                ncols = min(gkg * ROW, 980)
                hf = dbg.tile([P, ncols], f32, tag="hf")
                cp = nc.vector.tensor_copy(hf[:], hs[:, 0:ncols])
                bass._add_dep_helper(
                    cp.ins, flush.ins, sync=True, reason="debug drain")
                ov = out_d.ap().rearrange("(p q) j -> p (q j)", p=P)
                nc.sync.dma_start(out=ov[:, 0:ncols], in_=hf[:])
        nc.compile()
        return nc

    if "E" not in phases:
        # debug: dump h_sb (and inv_sb) into out
        with tile.TileContext(nc) as tc:
            ov = out_d.ap().rearrange("(p q) j -> p (q j)", p=P)
            nc.sync.dma_start(out=ov[:, 0:NT * NH], in_=h_sb.ap())
            nc.sync.dma_start(out=ov[:, NT * NH:NT * NH + NT], in_=inv_sb.ap())
        nc.compile()
        return nc

    with tile.TileContext(nc) as tc:
        with tc.tile_pool(name="hd", bufs=1) as hp, \
             tc.tile_pool(name="hdp", bufs=4, space="PSUM") as hpp:
            h3t = hp.tile([NH, NT * P], f32, tag="h3t")
            for t in range(NT):
                pst = hpp.tile([NH, P], f32, tag="pst")
                nc.tensor.transpose(
                    out=pst[:], in_=h_sb.ap()[:, t * NH:(t + 1) * NH],
                    identity=ident_f.ap())
                nc.vector.tensor_copy(h3t[:, t * P:(t + 1) * P], pst[:])
            lg = hp.tile([P, NT * NCO], f32, tag="lg")
            for t in range(NT):
                psl = hpp.tile([P, NCO], f32, tag="psl")
                nc.tensor.matmul(
                    out=psl[:], lhsT=h3t[:, t * P:(t + 1) * P], rhs=w4_sb.ap(),
                    start=True, stop=True)
                nc.vector.tensor_add(
                    lg[:, t * NCO:(t + 1) * NCO], psl[:], b4_sb.ap())
            lg3 = lg[:].rearrange("p (t j) -> p t j", t=NT)
            nm = hp.tile([P, NT], f32, tag="hnm")
            nc.vector.reduce_max(nm[:], lg3, axis=AX, negate=True)
            nc.vector.tensor_add(
                lg3, lg3, nm[:][:, :, None].to_broadcast([P, NT, NCO]))
            ex = hp.tile([P, NT * NCO], f32, tag="ex")
            nc.scalar.activation(ex[:], lg[:], AF.Exp)
            s = hp.tile([P, NT], f32, tag="hs_sum")
            nc.vector.reduce_sum(
                s[:], ex[:].rearrange("p (t j) -> p t j", t=NT), axis=AX)
            ls = hp.tile([P, NT], f32, tag="ls")
            nc.scalar.activation(ls[:], s[:], AF.Ln)
            nc.vector.tensor_sub(
                lg3, lg3, ls[:][:, :, None].to_broadcast([P, NT, NCO]))
            nc.sync.dma_start(
                out=out_d[:, :].rearrange("(t p) j -> p t j", p=P),
                in_=lg3)

    nc.compile()
    return nc


# --------------------------------------------------------------------------
# entry point
# --------------------------------------------------------------------------

def run(cfg: Cfg, inputs: dict, trace: bool = False):
    from concourse import bass_utils

    x = np.asarray(inputs["x"], np.float32)
    edge_index = np.asarray(inputs["edge_index"])
    W1 = np.asarray(inputs["W1"], np.float32)
    b1 = np.asarray(inputs["b1"], np.float32)
    W4 = np.asarray(inputs["W4"], np.float32)
    b4 = np.asarray(inputs["b4"], np.float32)
    beta3 = np.asarray(inputs["beta3"], np.float32)

    import ml_dtypes

    per_core, kg, order_c = preprocess(cfg, x, edge_index)
    nc = build_kernel(cfg, kg)

    P, KC, NH = cfg.P, cfg.kchunks, cfg.nh
    w1p = np.zeros((cfg.f_pad, NH), np.float32)
    w1p[:cfg.f_in] = W1
    w1p = np.ascontiguousarray(
        w1p.reshape(KC, P, NH).transpose(1, 0, 2).reshape(P, KC * NH)
    ).astype(ml_dtypes.bfloat16)
    b1r = np.ascontiguousarray(np.broadcast_to(b1[None, :], (P, NH)))
    b4r = np.ascontiguousarray(np.broadcast_to(b4[None, :], (P, cfg.nc_out)))
    ber = np.ascontiguousarray(np.broadcast_to(beta3[None, :], (P, 1)))

    in_maps = []
    for c in range(cfg.n_cores):
        in_maps.append({
            "xt": per_core[c]["xt"].astype(ml_dtypes.bfloat16),
            "idx": per_core[c]["idx"],
            "w1p": w1p, "b1r": b1r, "w4r": np.ascontiguousarray(W4),
            "b4r": b4r, "beta3r": ber,
        })

    res = bass_utils.run_bass_kernel_spmd(
        nc, in_maps, core_ids=list(range(cfg.n_cores)), trace=trace)

    out = np.empty((cfg.n_nodes, cfg.nc_out), np.float32)
    for c in range(cfg.n_cores):
        oc = np.asarray(res.results[c]["out"])[:cfg.npc_raw]
        out[c * cfg.npc_raw + order_c[c]] = oc
    return out, res


def kernel(**inputs) -> np.ndarray:
    out, _ = run(FULL, inputs, trace=False)
    return out

